# revision 25
# baseline (speedup 1.0000x reference)
"""Trainium2 Bass kernel for nn_MultiHeadAttention (B=2, S=2048, D=1024, H=16).

Sharding: 8 cores = 2 batches x 4 head-groups (4 heads per core, tensor
parallel over heads). Each core computes, for its batch b and its 4 heads:
  QT/KT = (x @ W.T).T projections in transposed layout [256, 2048]
  V     = value @ wv.T in normal layout, augmented with a ones column (Z trick)
  E^T   = exp(scoresT) tiles [k,q] directly from matmul (no max subtraction;
          scores are O(1) for this module so exp is safe, and masked entries
          use a multiplicative 0/1 mask so they are exactly 0)
  outT  = V_aug.T @ E^T accumulated over k tiles -> row 64 carries Z = sum(E)
  ffT   = wff_rows-partial @ (attn_outT * 1/Z) + bff/4   as [1024, 2048]
Host sums the 4 partial ffT per batch and transposes back.

Matmul chain runs in bf16 (fp32 PSUM accumulation); the Z-broadcast
rank-1 matmuls run in float32r to keep the softmax normalization at
near-fp32 precision. Phases are interleaved per 512-token group so PE
stays busy while DMA streams the next group's activations.
"""

import sys

sys.path.insert(0, "/opt/trn_rl_repo")

import ml_dtypes
import numpy as np

import concourse.bass as bass
import concourse.mybir as mybir
import concourse.tile as tile
from concourse import bacc
from concourse.bass_utils import run_bass_kernel_spmd

P = 128
B, S, D, H = 2, 2048, 1024, 16
DH = D // H  # 64
NCORES = 8
GPB = NCORES // B  # cores (head groups) per batch = 4
HPC = H // GPB  # heads per core = 4
HD = HPC * DH  # projected cols per core = 256
F32 = mybir.dt.float32
F32R = mybir.dt.float32r
BF16 = mybir.dt.bfloat16
QGW = 512  # q-group width (psum free dim)
AF = mybir.ActivationFunctionType
NPBF16 = ml_dtypes.bfloat16

_PROG_CACHE: dict = {}


def build_program(variant: str, use_bias: bool, s=S, d=D, hpc=HPC,
                  n_devices=NCORES):
    """variant: 'causal' | 'dense' | 'generic'. Returns compiled Bacc."""
    assert variant in ("causal", "dense", "generic")
    kc_n = d // P           # contraction chunks over model dim
    tt = s // P             # token tiles
    hd = hpc * DH           # per-core projected width
    dc_n = hd // P          # dout chunks for QT/KT (and hd chunks for ff)
    tg_n = s // QGW         # token/q groups
    tpg = QGW // P          # token tiles per group (4)
    zw = hpc * QGW          # z columns per qg-pair tile

    nc = bacc.Bacc("TRN2", target_bir_lowering=False, debug=False,
                   num_devices=n_devices)

    def din(name, shape, dt=BF16):
        return nc.dram_tensor(name, list(shape), dt, kind="ExternalInput").ap()

    xqT = din("xqT", (d, s))
    xkT = din("xkT", (d, s))
    xvT = din("xvT", (d, s))
    wqT = din("wqT", (d, hd))   # pre-scaled by 1/sqrt(DH) on host
    wkT = din("wkT", (d, hd))
    wvT = din("wvT", (d, hd))
    wffT = din("wffT", (hd, d))
    if use_bias:
        bq = din("bq", (hd,), F32)   # pre-scaled by 1/sqrt(DH) on host
        bk = din("bk", (hd,), F32)
        bv = din("bv", (1, hd))
        bffq = din("bffq", (d,), F32)    # bff / GPB
        onesb = din("onesb", (1, P))
    if variant == "causal":
        dmask = din("dmask", (P, P))  # [k, q]: 1 if k <= q else 0
    if variant == "generic":
        mbT = din("mbT", (s, s), F32)  # mask[b,0].T * -1e9, [k, q] layout
    outT = nc.dram_tensor("outT", [d, s], F32, kind="ExternalOutput").ap()
    zdr = nc.dram_tensor("zdr", [tg_n, hpc * QGW], F32).ap()

    with tile.TileContext(nc) as tc:
        with (
            nc.allow_low_precision(reason="bf16 matmul chain; psum stays fp32"),
            tc.tile_pool(name="consts", bufs=1) as consts,
            tc.tile_pool(name="xin", bufs=1) as xin,
            tc.tile_pool(name="acts", bufs=1) as acts,
            tc.tile_pool(name="epool", bufs=6) as epool,
            tc.tile_pool(name="opool", bufs=4) as opool,
            tc.tile_pool(name="ps", bufs=1, space="PSUM") as ps,
        ):
            # ---- constant / weight loads ----
            wq_sb = consts.tile([P, kc_n, hd], BF16, tag="wq")
            wk_sb = consts.tile([P, kc_n, hd], BF16, tag="wk")
            wv_sb = consts.tile([P, kc_n, hd], BF16, tag="wv")
            wff_sb = consts.tile([P, dc_n, d], BF16, tag="wff")
            nc.sync.dma_start(wq_sb[:], wqT.rearrange("(c p) m -> p c m", p=P))
            nc.sync.dma_start(wk_sb[:], wkT.rearrange("(c p) m -> p c m", p=P))
            nc.sync.dma_start(wv_sb[:], wvT.rearrange("(c p) m -> p c m", p=P))
            nc.sync.dma_start(wff_sb[:], wffT.rearrange("(c p) m -> p c m", p=P))
            if use_bias:
                bq_sb = consts.tile([P, dc_n], F32, tag="bq")
                bk_sb = consts.tile([P, dc_n], F32, tag="bk")
                nc.sync.dma_start(bq_sb[:], bq.rearrange("(c p) -> p c", p=P))
                nc.sync.dma_start(bk_sb[:], bk.rearrange("(c p) -> p c", p=P))
                bv_sb = consts.tile([1, hd], BF16, tag="bv")
                nc.sync.dma_start(bv_sb[:], bv[:])
                bffq_sb = consts.tile([P, kc_n], F32, tag="bffq")
                nc.sync.dma_start(bffq_sb[:],
                                  bffq.rearrange("(c p) -> p c", p=P))
                onesb_sb = consts.tile([1, P], BF16, tag="onesb")
                nc.sync.dma_start(onesb_sb[:], onesb[:])
            if variant == "causal":
                dmask_sb = consts.tile([P, P], BF16, tag="dmask")
                nc.sync.dma_start(dmask_sb[:], dmask[:])

            # per-group activation tiles (split so the scheduler can
            # pipeline groups without whole-tile false dependencies)
            qT_g = [acts.tile([P, dc_n, QGW], BF16, tag=f"qT{g}",
                              name=f"qT_{g}") for g in range(tg_n)]
            kT_g = [acts.tile([P, dc_n, QGW], BF16, tag=f"kT{g}",
                              name=f"kT_{g}") for g in range(tg_n)]
            va_g = [acts.tile([P, tpg, hpc * (DH + 1)], BF16, tag=f"va{g}",
                              name=f"va_{g}") for g in range(tg_n)]
            at_g = [acts.tile([P, dc_n, QGW], BF16, tag=f"at{g}",
                              name=f"at_{g}") for g in range(tg_n)]
            z_q = [acts.tile([1, zw], F32, tag=f"z{g}", name=f"z_{g}")
                   for g in range(tg_n)]
            zi_q = [acts.tile([1, zw], F32, tag=f"zi{g}", name=f"zi_{g}")
                    for g in range(tg_n)]
            zb_q = [acts.tile([P, zw], F32, tag=f"zb{g}", name=f"zb_{g}")
                    for g in range(tg_n)]

            def proj_qk(tg, w_sb, x_dram, b_sb, dest):
                xts = []
                for kc in range(kc_n):
                    xt = xin.tile([P, QGW], BF16, tag="xstream", bufs=12,
                                  name=f"xt_{tg}_{kc}")
                    nc.sync.dma_start(
                        xt[:],
                        x_dram[kc * P:(kc + 1) * P, tg * QGW:(tg + 1) * QGW])
                    xts.append(xt)
                for dc in range(dc_n):
                    pp = ps.tile([P, QGW], F32, tag="pacc", bufs=2,
                                 name=f"pp_{tg}_{dc}")
                    for kc in range(kc_n):
                        nc.tensor.matmul(
                            pp[:],
                            lhsT=w_sb[:, kc, dc * P:(dc + 1) * P],
                            rhs=xts[kc][:],
                            start=(kc == 0),
                            stop=(kc == kc_n - 1),
                        )
                    if use_bias:
                        nc.scalar.activation(dest[:, dc, :], pp[:],
                                             AF.Identity,
                                             bias=b_sb[:, dc:dc + 1])
                    else:
                        nc.vector.tensor_copy(dest[:, dc, :], pp[:])

            def proj_v(tg):
                nc.gpsimd.memset(
                    va_g[tg].rearrange("p t (h e) -> p t h e",
                                       e=DH + 1)[:, :, :, DH], 1.0)
                for ti in range(tpg):
                    t = tg * tpg + ti
                    xvt = xin.tile([P, kc_n, P], BF16, tag="xvstream", bufs=4,
                                   name=f"xvt_{t}")
                    nc.sync.dma_start(
                        xvt[:],
                        xvT[:, t * P:(t + 1) * P].rearrange(
                            "(c p) t -> p c t", p=P))
                    vp = ps.tile([P, QGW], F32, tag="acc", bufs=2,
                                 name=f"vp_{t}")
                    if use_bias:
                        nc.tensor.matmul(vp[:, :hd], lhsT=onesb_sb[0:1, :],
                                         rhs=bv_sb[:, :], start=True,
                                         stop=False)
                    for kc in range(kc_n):
                        nc.tensor.matmul(
                            vp[:, :hd],
                            lhsT=xvt[:, kc, :],
                            rhs=wv_sb[:, kc, :],
                            start=(kc == 0 and not use_bias),
                            stop=(kc == kc_n - 1),
                        )
                    nc.vector.tensor_copy(
                        va_g[tg][:, ti].rearrange(
                            "p (h e) -> p h e", e=DH + 1)[:, :, :DH],
                        vp[:, :hd].rearrange("p (h e) -> p h e", e=DH))

            def attention(qg):
                kmax = (qg + 1) * tpg if variant == "causal" else tt
                PW = 2  # score tiles batched per exp
                nquad = kmax // PW
                for h in range(hpc):
                    po = (h * DH) % P
                    dch = (h * DH) // P
                    op = ps.tile([P, QGW], F32, tag="acc", bufs=2,
                                 name=f"op_{h}_{qg}")
                    ets = [None] * nquad

                    def emit_scores(qd):
                        sp = ps.tile([P, PW * QGW], F32, tag="mmw", bufs=2,
                                     name=f"sp_{h}_{qg}_{qd}")
                        for j in range(PW):
                            kt = qd * PW + j
                            off = (max(0, kt * P - qg * QGW)
                                   if variant == "causal" else 0)
                            kg, kx = kt // tpg, kt % tpg
                            kh = kT_g[kg][po:po + DH, dch,
                                          kx * P:(kx + 1) * P]
                            nc.tensor.matmul(
                                sp[:, j * QGW + off:(j + 1) * QGW],
                                lhsT=kh,
                                rhs=qT_g[qg][po:po + DH, dch, off:],
                                start=True,
                                stop=True,
                            )
                            if variant == "generic":
                                mb_sb = xin.tile([P, QGW], F32, tag="mstream",
                                                 bufs=4,
                                                 name=f"mb_{h}_{qg}_{kt}")
                                nc.sync.dma_start(
                                    mb_sb[:],
                                    mbT[kt * P:(kt + 1) * P,
                                        qg * QGW:(qg + 1) * QGW])
                                nc.vector.tensor_add(
                                    sp[:, j * QGW:(j + 1) * QGW],
                                    sp[:, j * QGW:(j + 1) * QGW], mb_sb[:])
                        et = epool.tile([P, PW * QGW], BF16, tag="etile",
                                        name=f"et_{h}_{qg}_{qd}")
                        offs = [(max(0, (qd * PW + j) * P - qg * QGW)
                                 if variant == "causal" else 0)
                                for j in range(PW)]
                        if not any(offs):
                            nc.scalar.activation(et[:], sp[:], AF.Exp)
                        else:
                            for j in range(PW):
                                o = j * QGW + offs[j]
                                nc.scalar.activation(
                                    et[:, o:(j + 1) * QGW],
                                    sp[:, o:(j + 1) * QGW], AF.Exp)
                        if variant == "causal":
                            for j in range(PW):
                                kt = qd * PW + j
                                off = kt * P - qg * QGW
                                if off < 0:
                                    continue
                                if off:
                                    nc.gpsimd.memset(
                                        et[:, j * QGW:j * QGW + off], 0.0)
                                nc.vector.tensor_mul(
                                    et[:, j * QGW + off:j * QGW + off + P],
                                    et[:, j * QGW + off:j * QGW + off + P],
                                    dmask_sb[:])
                        ets[qd] = et

                    def emit_av(qd):
                        et = ets[qd]
                        for j in range(PW):
                            kt = qd * PW + j
                            kg, kx = kt // tpg, kt % tpg
                            nc.tensor.matmul(
                                op[:DH + 1, :],
                                lhsT=va_g[kg][:, kx, h * (DH + 1):
                                              (h + 1) * (DH + 1)],
                                rhs=et[:, j * QGW:(j + 1) * QGW],
                                start=(kt == 0),
                                stop=(kt == kmax - 1),
                            )
                        ets[qd] = None

                    emit_scores(0)
                    for qd in range(1, nquad):
                        emit_scores(qd)
                        emit_av(qd - 1)
                    emit_av(nquad - 1)
                    nc.vector.tensor_copy(
                        at_g[qg][po:po + DH, dch, :], op[:DH, :])
                    nc.vector.tensor_copy(
                        z_q[qg][0:1, h * QGW:(h + 1) * QGW],
                        op[DH:DH + 1, :])
                # zinv for the whole group, then broadcast to 128 partitions
                # via a DRAM bounce (engines cannot partition-broadcast)
                nc.vector.reciprocal_approx_fast(zi_q[qg][:], z_q[qg][:])
                nc.sync.dma_start(zdr[qg:qg + 1, :], zi_q[qg][:])
                nc.sync.dma_start(zb_q[qg][:],
                                  zdr[qg:qg + 1, :].to_broadcast([P, zw]))

            def norm_ff(qg):
                for h in range(hpc):
                    dc = (h * DH) // P
                    po = (h * DH) % P
                    nc.vector.tensor_mul(
                        at_g[qg][po:po + DH, dc, :],
                        at_g[qg][po:po + DH, dc, :],
                        zb_q[qg][po:po + DH, h * QGW:(h + 1) * QGW],
                    )
                for nck in range(kc_n):
                    fp = ps.tile([P, QGW], F32, tag="acc", bufs=2,
                                 name=f"fp_{nck}_{qg}")
                    for dc in range(dc_n):
                        nc.tensor.matmul(
                            fp[:],
                            lhsT=wff_sb[:, dc, nck * P:(nck + 1) * P],
                            rhs=at_g[qg][:, dc, :],
                            start=(dc == 0),
                            stop=(dc == dc_n - 1),
                        )
                    ot = opool.tile([P, QGW], F32, tag="otile",
                                    name=f"ot_{nck}_{qg}")
                    if use_bias:
                        nc.scalar.activation(ot[:], fp[:], AF.Identity,
                                             bias=bffq_sb[:, nck:nck + 1])
                    else:
                        nc.scalar.copy(ot[:], fp[:])
                    nc.sync.dma_start(
                        outT[nck * P:(nck + 1) * P,
                             qg * QGW:(qg + 1) * QGW], ot[:])

            # ---- schedule over token groups ----
            # causal: attention(qg) only needs k/v groups <= qg, so proj and
            # attention interleave per group. dense/generic attend over the
            # full sequence: all projections first.
            if variant == "causal":
                for tg in range(tg_n):
                    proj_qk(tg, wq_sb, xqT, bq_sb if use_bias else None,
                            qT_g[tg])
                    proj_qk(tg, wk_sb, xkT, bk_sb if use_bias else None,
                            kT_g[tg])
                    proj_v(tg)
                    attention(tg)
                    if tg > 0:
                        norm_ff(tg - 1)
                norm_ff(tg_n - 1)
            else:
                for tg in range(tg_n):
                    proj_qk(tg, wq_sb, xqT, bq_sb if use_bias else None,
                            qT_g[tg])
                    proj_qk(tg, wk_sb, xkT, bk_sb if use_bias else None,
                            kT_g[tg])
                    proj_v(tg)
                for qg in range(tg_n):
                    attention(qg)
                    if qg > 0:
                        norm_ff(qg - 1)
                norm_ff(tg_n - 1)

    nc.compile()
    return nc


def _classify_mask(mask: np.ndarray) -> str:
    m = np.asarray(mask)[:, 0]  # [B, S, S]
    if not m.any():
        return "dense"
    s = m.shape[-1]
    causal = np.triu(np.ones((s, s), dtype=m.dtype), k=1)
    if all(np.array_equal(m[b], causal) for b in range(m.shape[0])):
        return "causal"
    return "generic"


def _bf(x):
    return np.ascontiguousarray(np.ascontiguousarray(x).astype(NPBF16))


def _make_in_maps(variant, query, key, value, mask, wq, bq, wk, bk, wv, bv,
                  wff, bff, use_bias):
    scale = np.float32(1.0 / np.sqrt(np.float32(DH)))
    wqTs = _bf((wq * scale).T)
    wkT = _bf(wk.T)
    wvT = _bf(wv.T)
    wffT = _bf(wff.T)

    qT = [_bf(query[b].T) for b in range(B)]
    kT = [_bf(key[b].T) for b in range(B)]
    vT = [_bf(value[b].T) for b in range(B)]
    mbT = None
    if variant == "generic":
        mbT = [np.ascontiguousarray(mask[b, 0].T * np.float32(-1e9))
               for b in range(B)]

    dmask = np.tril(np.ones((P, P), np.float32)).T

    in_maps = []
    for c in range(NCORES):
        b, hg = c // GPB, c % GPB
        sl = slice(hg * HD, (hg + 1) * HD)
        m = {
            "xqT": qT[b], "xkT": kT[b], "xvT": vT[b],
            "wqT": np.ascontiguousarray(wqTs[:, sl]),
            "wkT": np.ascontiguousarray(wkT[:, sl]),
            "wvT": np.ascontiguousarray(wvT[:, sl]),
            "wffT": np.ascontiguousarray(wffT[sl, :]),
        }
        if use_bias:
            m["bq"] = np.ascontiguousarray((bq * scale)[sl]).astype(np.float32)
            m["bk"] = np.ascontiguousarray(bk[sl]).astype(np.float32)
            m["bv"] = _bf(bv[sl])[None, :]
            m["bffq"] = (bff / GPB).astype(np.float32)
            m["onesb"] = np.ones((1, P), NPBF16)
        if variant == "causal":
            m["dmask"] = _bf(dmask)
        if variant == "generic":
            m["mbT"] = mbT[b]
        in_maps.append(m)
    return in_maps


def kernel(**inputs) -> np.ndarray:
    query = np.ascontiguousarray(inputs["query"], dtype=np.float32)
    key = np.ascontiguousarray(inputs["key"], dtype=np.float32)
    value = np.ascontiguousarray(inputs["value"], dtype=np.float32)
    mask = np.asarray(inputs["mask"], dtype=np.float32)
    wq = np.asarray(inputs["wq"], np.float32)
    bq = np.asarray(inputs["bq"], np.float32)
    wk = np.asarray(inputs["wk"], np.float32)
    bk = np.asarray(inputs["bk"], np.float32)
    wv = np.asarray(inputs["wv"], np.float32)
    bv = np.asarray(inputs["bv"], np.float32)
    wff = np.asarray(inputs["wff"], np.float32)
    bff = np.asarray(inputs["bff"], np.float32)

    variant = _classify_mask(mask)
    use_bias = bool(bq.any() or bk.any() or bv.any() or bff.any())
    pkey = (variant, use_bias)
    if pkey not in _PROG_CACHE:
        _PROG_CACHE[pkey] = build_program(variant, use_bias)
    nc = _PROG_CACHE[pkey]

    in_maps = _make_in_maps(variant, query, key, value, mask, wq, bq, wk, bk,
                            wv, bv, wff, bff, use_bias)
    res = run_bass_kernel_spmd(nc, in_maps, core_ids=list(range(NCORES)))
    out = np.empty((B, S, D), np.float32)
    for b in range(B):
        acc = res.results[b * GPB]["outT"].astype(np.float32)
        for g in range(1, GPB):
            acc = acc + res.results[b * GPB + g]["outT"]
        out[b] = acc.T
    return out


if __name__ == "__main__":
    import reference

    inputs = {k: np.asarray(v) for k, v in reference.setup_inputs().items()}
    out = kernel(**inputs)
    print("kernel out:", out.shape, out.dtype)
